# revision 23
# baseline (speedup 1.0000x reference)
"""Multi-head attention (raw-reshape variant) on 8 trn2 NeuronCores.

Shapes: B=2, S=2048, D=1024, H=16, dh=64.  The reference uses a raw
reshape (B,S,D)->(B,H,S,dh), so head h only sees projected rows
[128h, 128h+128).  Core c handles b=c//4 and the 4 heads of seq-block
c%4.  No collectives.

All data tensors are fp16: softmax-weight quantization noise passes
through to the output at full strength, so fp8 anywhere costs ~3.6%+
output error -- over the 2e-2 gate.

v4 (from the v3 baseline, ~259-265us; this version measures
~235-260us, run-to-run clock-governor variance is +-20us):
  - mask arrives as fp16 via DMA (pre-scaled by e^-4 on the host);
    kills the 16 DVE u8->fp16 casts that inflated the attention-step
    cadence from ~1.07us to ~1.9us, plus the tile_wait_until hack.
    Mask DMAs MUST stay on the gpsimd queue: HWDGE dma_starts block
    the sync/scalar queues until the transfer lands, which starves
    the scatter casts and with them the psum recycling K proj needs.
  - Q/K projections j-outer (tolerate per-chunk DMA arrival; tt-outer
    stalls waiting for all 8 chunks), V (p,oc)-outer so vaug[0] is
    ready first; scatter casts split across DVE+ACT.
  - wo emit spread as 4 bursts of 4 matmuls at t=2,4,6,8 of the next
    block instead of one 16-matmul burst.
  - PV drains at lag 3 (not 5): shorter end-of-block drain, ~1.2us
    slack over the exp->mul chain.
Per-step pipeline unchanged: St = Kt.T @ Qt (K=64) -> exp on ACT ->
mask-mul on DVE -> PV accumulate (K=128, denominator rides along in
the vaug ones-columns); reciprocal+repack on DVE; one wo load.
Known floors per core: PE ~164us at 2.4GHz for the 768 matmuls (the
governor rarely sustains 2.4; St runs K=64 = half-PE inherently),
ACT exp 128 x ~1.11us = 143us.  fp8 anywhere fails the 2e-2 gate
(softmax-weight noise passes through at full strength).
"""

import numpy as np

import concourse.bass as bass
import concourse.mybir as mybir
import concourse.tile as tile
from concourse import bacc
from concourse.bass_utils import run_bass_kernel_spmd

F32 = mybir.dt.float32
F16 = mybir.dt.float16

B, S, D, H, DH = 2, 2048, 1024, 16, 64
N_CORES = 8
CORE_ROWS = 512
N_PAIRS = 4
WSCALE = 16.0
EXP_SCALE = 0.125 / (WSCALE * WSCALE)
MASK_SHIFT = float(np.exp(-4.0))

_NC = None


def _build_program():
    nc = bacc.Bacc()

    # per contraction chunk j: [w chunk (1024 dm) | x chunk (512 rows)]
    qasm = nc.dram_tensor("qasm", [8, 128, 1536], F16, kind="ExternalInput")
    kasm = nc.dram_tensor("kasm", [8, 128, 1536], F16, kind="ExternalInput")
    vasm = nc.dram_tensor("vasm", [8, 128, 1536], F16, kind="ExternalInput")
    wodr = nc.dram_tensor("wodr", [128, 8192], F16, kind="ExternalInput")
    # maskf[t, j, qcol]: fp16 keep-mask * e^-4 for key chunk t (rows
    # k'=16j+t), q columns in (c,r)-permuted order
    maskf_d = nc.dram_tensor("maskf", [16, 128, S], F16, kind="ExternalInput")
    out_d = nc.dram_tensor("out", [CORE_ROWS, D], F32, kind="ExternalOutput")

    with tile.TileContext(nc) as tc:
        with tc.tile_pool(name="persist", bufs=1) as persist:
            qt_all = persist.tile([128, 2 * S], F16, tag="qt", name="qt")
            kt_all = persist.tile([128, 2 * S], F16, tag="kt", name="kt")
            vaug = [persist.tile([128, 2048], F16, tag=f"vaug{p}", name=f"vaug{p}")
                    for p in range(N_PAIRS)]
            # stack2[p]: [64tp+d, 512qh+128tt+r] = 16*O^T[d, q''] with
            # q'' = 1024qh + 128(2tt+tp) + r, tt in [0,4)
            stack2 = [persist.tile([128, 1024], F16, tag=f"stk{p}", name=f"stk{p}")
                      for p in range(N_PAIRS)]
            wo_sb = persist.tile([128, 8192], F16, tag="wo", name="wo")
            maskf_sb = [persist.tile([128, S], F16, tag=f"mask{t}", name=f"mask{t}")
                        for t in range(16)]

            # ---------------- Phase 1: projections ----------------
            with tc.tile_pool(name="asm_sb", bufs=1) as asmp:
                qsb = [asmp.tile([128, 1536], F16, tag=f"qsb{j}", name=f"qsb{j}")
                       for j in range(8)]
                ksb = [asmp.tile([128, 1536], F16, tag=f"ksb{j}", name=f"ksb{j}")
                       for j in range(8)]
                vsb = [asmp.tile([128, 1536], F16, tag=f"vsb{j}", name=f"vsb{j}")
                       for j in range(8)]
                # DMA priority: q (PE starts on chunk 0), k, early masks,
                # v, wo, remaining masks.
                for j in range(8):
                    (nc.sync if j % 2 == 0 else nc.scalar).dma_start(
                        out=qsb[j][:, :], in_=qasm[j])
                for j in range(8):
                    (nc.sync if j % 2 == 1 else nc.scalar).dma_start(
                        out=ksb[j][:, :], in_=kasm[j])
                # masks go on the gpsimd queue only: HWDGE dma_starts on
                # sync/scalar block those queues until the transfer lands,
                # which would stall the scatter casts (and with them the
                # psum recycling that K proj needs)
                # gate the gpsimd DMA batch on the last k chunk so q/k get
                # the full DMA bus while the PE is consuming them; v isn't
                # needed until ~50us, masks until the attention stream
                scr = asmp.tile([128, 8], F16, tag="scr", name="scr")
                nc.gpsimd.tensor_copy(scr[:, :], ksb[7][:, 1528:1536])
                for j in range(8):
                    nc.gpsimd.dma_start(out=vsb[j][:, :], in_=vasm[j])
                nc.sync.dma_start(out=wo_sb[:, :], in_=wodr[:, :])
                for t in range(16):
                    nc.gpsimd.dma_start(out=maskf_sb[t][:, :], in_=maskf_d[t])
                for p in range(N_PAIRS):
                    va3 = vaug[p][:, :].rearrange("p (t c) -> p t c", c=128)
                    nc.gpsimd.memset(va3[:, :, 0:64], 1.0)

                def scatter_qk(ps, dst_all, tt, engines):
                    # scatter-cast: psum[64sub+d, 256g+128hp+r]
                    #   -> dst[64hp+d, 2048g+128(2tt+sub)+r]
                    dst4 = dst_all[:, :].rearrange(
                        "p (g t r) -> p g t r", g=2, t=16)
                    i = 0
                    for sub in range(2):
                        s4 = ps[64 * sub:64 * (sub + 1), :].rearrange(
                            "p (g h r) -> p g h r", g=2, h=2)
                        for hp in range(2):
                            d_ap = dst4[64 * hp:64 * (hp + 1), :, 2 * tt + sub, :]
                            s_ap = s4[:, :, hp, :]
                            eng = engines[i % len(engines)]
                            i += 1
                            if eng is nc.scalar:
                                nc.scalar.activation(
                                    d_ap, s_ap,
                                    mybir.ActivationFunctionType.Copy)
                            else:
                                eng.tensor_copy(d_ap, s_ap)

                with tc.tile_pool(name="qk_ps", bufs=8, space="PSUM") as qkps:
                    # Q: j-outer over 8 concurrent psum banks -- the first
                    # matmul depends only on the first DMA chunk
                    psq = [qkps.tile([128, 512], F32, tag="qk", name=f"psq{tt}")
                           for tt in range(8)]
                    for j in range(8):
                        for tt in range(8):
                            nc.tensor.matmul(
                                psq[tt][:, :],
                                lhsT=qsb[j][:, 128 * tt:128 * (tt + 1)],
                                rhs=qsb[j][:, 1024:1536],
                                start=(j == 0), stop=(j == 7))
                            if j == 7:
                                scatter_qk(psq[tt], qt_all, tt,
                                           (nc.vector, nc.scalar))
                    # K: j-outer like Q (tolerates per-chunk DMA arrival;
                    # tt-outer needs all 8 chunks up front and stalls)
                    psk = [qkps.tile([128, 512], F32, tag="qk", name=f"psk{tt}")
                           for tt in range(8)]
                    for j in range(8):
                        for tt in range(8):
                            nc.tensor.matmul(
                                psk[tt][:, :],
                                lhsT=ksb[j][:, 128 * tt:128 * (tt + 1)],
                                rhs=ksb[j][:, 1024:1536],
                                start=(j == 0), stop=(j == 7))
                            if j == 7:
                                scatter_qk(psk[tt], kt_all, tt,
                                           (nc.scalar, nc.vector))

                    # V: (p,oc)-outer so vaug[0] is ready first
                    psv = [qkps.tile([128, 512], F32, tag="qk",
                                     name=f"psv{i}") for i in range(8)]
                    for p in range(N_PAIRS):
                        d3 = vaug[p][:, :].rearrange("p (t c) -> p t c", c=128)
                        for oc in range(2):
                            for j in range(8):
                                nc.tensor.matmul(
                                    psv[2 * p + oc][:, :],
                                    lhsT=vsb[j][:, 1024 + 128 * p:1024 + 128 * (p + 1)],
                                    rhs=vsb[j][:, 512 * oc:512 * (oc + 1)],
                                    start=(j == 0), stop=(j == 7))
                            s3 = psv[2 * p + oc][:, :].rearrange(
                                "p (t c) -> p t c", c=64)
                            nc.vector.tensor_copy(
                                d3[:, 8 * oc:8 * (oc + 1), 64:128], s3)

            # ---------------- Phase 2: attention + output ----------------
            with tc.tile_pool(name="praw_p", bufs=3) as ppool, \
                 tc.tile_pool(name="pm_p", bufs=6) as pmpool, \
                 tc.tile_pool(name="norm", bufs=2) as npool, \
                 tc.tile_pool(name="outc", bufs=2) as opool, \
                 tc.tile_pool(name="st_ps", bufs=3, space="PSUM") as stps, \
                 tc.tile_pool(name="o_ps", bufs=1, space="PSUM") as ops:

                wo3 = wo_sb[:, :].rearrange("p (tt x) -> p tt x", tt=8)

                def emit_part(p, psF, part):
                    # 4 of the 16 wo matmuls (part in 0..3 = (qh, tt-pair))
                    qh, th = part // 2, part % 2
                    for tt in (2 * th, 2 * th + 1):
                        TT = 4 * qh + tt   # global t-pair = t//2
                        for oc in range(2):
                            nc.tensor.matmul(
                                psF[:, 512 * oc:512 * (oc + 1)],
                                lhsT=stack2[p][:, 512 * qh + 128 * tt:
                                               512 * qh + 128 * (tt + 1)],
                                rhs=wo3[:, TT, 512 * oc:512 * (oc + 1)],
                                start=(part == 0 and tt == 0),
                                stop=(part == 3 and tt == 3))
                    if part == 3:
                        # wodr carries 1/WSCALE so psF is final; copy+DMA
                        # in halves so the first DMA overlaps the second copy
                        osb = opool.tile([128, 1024], F32, tag="osb", name="osb")
                        nc.vector.tensor_copy(osb[:, 0:512], psF[:, 0:512])
                        nc.sync.dma_start(out=out_d[128 * p:128 * (p + 1), 0:512],
                                          in_=osb[:, 0:512])
                        nc.vector.tensor_copy(osb[:, 512:1024], psF[:, 512:1024])
                        nc.scalar.dma_start(
                            out=out_d[128 * p:128 * (p + 1), 512:1024],
                            in_=osb[:, 512:1024])

                pending_emit = [None]

                for p in range(N_PAIRS):
                    g, hp = p // 2, p % 2
                    lo, hi = 64 * hp, 64 * (hp + 1)
                    for qh in range(2):
                        psO = ops.tile([128, 1024], F32, tag="o", name="psO")
                        queue = []

                        def drain_one():
                            t, pm = queue.pop(0)
                            for sc in range(2):
                                nc.tensor.matmul(
                                    psO[:, 512 * sc:512 * (sc + 1)],
                                    lhsT=vaug[p][:, 128 * t:128 * (t + 1)],
                                    rhs=pm[:, 512 * sc:512 * (sc + 1)],
                                    start=(t == 0), stop=(t == 15))

                        for t in range(16):
                            stt = stps.tile([128, 1024], F32, tag="st", name="stt")
                            for sc in range(2):
                                nc.tensor.matmul(
                                    stt[:, 512 * sc:512 * (sc + 1)],
                                    lhsT=kt_all[lo:hi,
                                                2048 * g + 128 * t:2048 * g + 128 * (t + 1)],
                                    rhs=qt_all[lo:hi,
                                               2048 * g + 1024 * qh + 512 * sc:
                                               2048 * g + 1024 * qh + 512 * (sc + 1)],
                                    start=True, stop=True)
                            praw = ppool.tile([128, 1024], F16, tag="praw", name="praw")
                            nc.scalar.activation(praw[:, :], stt[:, :],
                                                 mybir.ActivationFunctionType.Exp,
                                                 scale=EXP_SCALE)
                            pm = pmpool.tile([128, 1024], F16, tag="pm", name="pm")
                            nc.vector.tensor_mul(pm[:, :], praw[:, :],
                                                 maskf_sb[t][:, 1024 * qh:1024 * (qh + 1)])
                            queue.append((t, pm))
                            if p == 3 and qh == 1 and t == 10:
                                # last pair: its qh0 stack2 half is already
                                # written, so emit those 8 wo matmuls inside
                                # this block; only qh1's 8 remain for the
                                # tail.  psF allocated late to limit stt
                                # pool starvation.
                                pending_emit[0] = (3, stps.tile(
                                    [128, 1024], F32, tag="st", name="psF"),
                                    0, {11: 0, 13: 1})
                            if pending_emit[0] is not None:
                                ep, epsF, done, sched = pending_emit[0]
                                # spread the 16 wo matmuls as bursts of 4
                                # so no single burst stalls the exp cadence
                                if sched.get(t) == done:
                                    emit_part(ep, epsF, done)
                                    pending_emit[0] = (ep, epsF, done + 1,
                                                       sched)
                                    if done + 1 == 4:
                                        pending_emit[0] = None
                            if len(queue) > 2:
                                drain_one()
                        while queue:
                            drain_one()

                        # psO[0:64] = den copies, psO[64:128] = 16*O^T
                        recip = npool.tile([64, 1024], F32, tag="rc", name="recip")
                        nc.vector.reciprocal_approx_fast(recip[:, :], psO[0:64, :])
                        tmpn = npool.tile([128, 1024], F16, tag="tn", name="tmpn")
                        nc.vector.tensor_mul(tmpn[64:128, :], psO[64:128, :],
                                             recip[:, :])
                        # repack to stack2: even t -> partitions 0:64,
                        # odd t -> 64:128; cols compress 128tq'+r -> 128tt+r
                        src3 = tmpn[64:128, :].rearrange(
                            "p (tt tp r) -> p tt tp r", tt=4, tp=2)
                        for tp in range(2):
                            nc.vector.tensor_copy(
                                stack2[p][64 * tp:64 * (tp + 1),
                                          512 * qh:512 * (qh + 1)],
                                src3[:, :, tp, :])
                        if qh == 1 and p < 3:
                            pending_emit[0] = (p, stps.tile([128, 1024], F32,
                                                            tag="st", name="psF"),
                                               0, {2: 0, 4: 1, 6: 2, 8: 3})
                if pending_emit[0] is not None:
                    ep, epsF, done, _ = pending_emit[0]
                    for part in range(done, 4):
                        emit_part(ep, epsF, part)

    nc.finalize()
    return nc


def build_in_maps(inputs):
    q = np.asarray(inputs["q"], dtype=np.float32)
    k = np.asarray(inputs["k"], dtype=np.float32)
    v = np.asarray(inputs["v"], dtype=np.float32)
    mask = np.asarray(inputs["mask"])
    w_q = np.asarray(inputs["w_q"], dtype=np.float32)
    w_k = np.asarray(inputs["w_k"], dtype=np.float32)
    w_v = np.asarray(inputs["w_v"], dtype=np.float32)
    w_o = np.asarray(inputs["w_o"], dtype=np.float32)

    wqT = np.ascontiguousarray(w_q.T) * WSCALE
    wkT = np.ascontiguousarray(w_k.T) * WSCALE
    wvT = np.ascontiguousarray(w_v.T) * WSCALE
    # 1/WSCALE (not WSCALE): folds the 1/WSCALE^2 rescale of the
    # wv*wo WSCALE factors into the weights, so psF needs no rescale
    wo16 = np.ascontiguousarray(w_o.T) * (1.0 / WSCALE)  # [dm, c']
    # wodr[64tp+d, 1024tt + c'] = wo16[64(2tt+tp)+d, c']
    wodr = np.ascontiguousarray(
        wo16.reshape(8, 2, 64, D).transpose(1, 2, 0, 3).reshape(128, 8 * D)
    ).astype(np.float16)

    maskf = []
    for b in range(B):
        mt = (~mask[b]).T.astype(np.float16) * np.float16(MASK_SHIFT)
        mp = mt.reshape(S, 128, 16).transpose(0, 2, 1).reshape(S, S)
        maskf.append(np.ascontiguousarray(
            mp.reshape(128, 16, S).transpose(1, 0, 2)))

    in_maps = []
    for c in range(N_CORES):
        b, sb = c // 4, c % 4
        rows = slice(CORE_ROWS * sb, CORE_ROWS * (sb + 1))
        xqT = np.ascontiguousarray(q[b, rows].T)
        xkT = np.ascontiguousarray(k[b, rows].T)
        xvT = np.ascontiguousarray(v[b, rows].T)

        def pack(wT, xT):
            wc = wT.reshape(8, 128, D)
            xc = xT.reshape(8, 128, CORE_ROWS)
            return np.ascontiguousarray(
                np.concatenate([wc, xc], axis=2)).astype(np.float16)

        in_maps.append({
            "qasm": pack(wqT, xqT),
            "kasm": pack(wkT, xkT),
            "vasm": pack(wvT, xvT),
            "wodr": wodr,
            "maskf": maskf[b],
        })
    return in_maps


def kernel(q, k, v, mask, w_q, w_k, w_v, w_o):
    global _NC
    if _NC is None:
        _NC = _build_program()

    in_maps = build_in_maps(dict(q=q, k=k, v=v, mask=mask,
                                 w_q=w_q, w_k=w_k, w_v=w_v, w_o=w_o))
    res = run_bass_kernel_spmd(_NC, in_maps, list(range(N_CORES))).results

    out = np.empty((B, S, D), dtype=np.float32)
    for c in range(N_CORES):
        b, sb = c // 4, c % 4
        out[b, CORE_ROWS * sb:CORE_ROWS * (sb + 1)] = res[c]["out"]
    return out


# revision 26
# speedup vs baseline: 1.0001x; 1.0001x over previous
"""Multi-head attention (raw-reshape variant) on 8 trn2 NeuronCores.

Shapes: B=2, S=2048, D=1024, H=16, dh=64.  The reference uses a raw
reshape (B,S,D)->(B,H,S,dh), so head h only sees projected rows
[128h, 128h+128).  Core c handles b=c//4 and the 4 heads of seq-block
c%4.  No collectives.

All data tensors are fp16: softmax-weight quantization noise passes
through to the output at full strength, so fp8 anywhere costs ~3.6%+
output error -- over the 2e-2 gate.

v4 (from the v3 baseline, ~259-265us; this version measures
~235-260us, run-to-run clock-governor variance is +-20us):
  - mask arrives as fp16 via DMA (pre-scaled by e^-4 on the host);
    kills the 16 DVE u8->fp16 casts that inflated the attention-step
    cadence from ~1.07us to ~1.9us, plus the tile_wait_until hack.
    Mask DMAs MUST stay on the gpsimd queue: HWDGE dma_starts block
    the sync/scalar queues until the transfer lands, which starves
    the scatter casts and with them the psum recycling K proj needs.
  - Q/K projections j-outer (tolerate per-chunk DMA arrival; tt-outer
    stalls waiting for all 8 chunks), V (p,oc)-outer so vaug[0] is
    ready first; scatter casts split across DVE+ACT.
  - wo emit spread as 4 bursts of 4 matmuls at t=2,4,6,8 of the next
    block instead of one 16-matmul burst; the last pair emits its qh0
    half inside its own final block (psF allocated at t=10 to limit
    stt-pool starvation), so only 8 wo matmuls remain in the tail.
  - PV drains at lag 3 (not 5): shorter end-of-block drain, ~1.2us
    slack over the exp->mul chain.
  - wodr carries 1/WSCALE (not WSCALE), folding the 1/256 rescale into
    the weights: the final psum->sbuf move is a plain copy, split per
    512-col half so each output DMA overlaps the next copy.
  - gpsimd DMA batch (v + masks) gated on the last k chunk via a tiny
    dependency copy: q/k get the full DMA bus while the PE consumes
    them (v needed ~50us, masks ~66us).
Per-step pipeline unchanged: St = Kt.T @ Qt (K=64) -> exp on ACT ->
mask-mul on DVE -> PV accumulate (K=128, denominator rides along in
the vaug ones-columns); reciprocal+repack on DVE; one wo load.
Known floors per core: PE ~164us at 2.4GHz for the 768 matmuls (the
governor rarely sustains 2.4; St runs K=64 = half-PE inherently),
ACT exp 128 x ~1.11us = 143us.  fp8 anywhere fails the 2e-2 gate
(softmax-weight noise passes through at full strength).
"""

import numpy as np

import concourse.bass as bass
import concourse.mybir as mybir
import concourse.tile as tile
from concourse import bacc
from concourse.bass_utils import run_bass_kernel_spmd

F32 = mybir.dt.float32
F16 = mybir.dt.float16

B, S, D, H, DH = 2, 2048, 1024, 16, 64
N_CORES = 8
CORE_ROWS = 512
N_PAIRS = 4
WSCALE = 16.0
EXP_SCALE = 0.125 / (WSCALE * WSCALE)
MASK_SHIFT = float(np.exp(-4.0))

_NC = None


def _build_program():
    nc = bacc.Bacc()

    # per contraction chunk j: [w chunk (1024 dm) | x chunk (512 rows)]
    qasm = nc.dram_tensor("qasm", [8, 128, 1536], F16, kind="ExternalInput")
    kasm = nc.dram_tensor("kasm", [8, 128, 1536], F16, kind="ExternalInput")
    vasm = nc.dram_tensor("vasm", [8, 128, 1536], F16, kind="ExternalInput")
    wodr = nc.dram_tensor("wodr", [128, 8192], F16, kind="ExternalInput")
    # maskf[t, j, qcol]: fp16 keep-mask * e^-4 for key chunk t (rows
    # k'=16j+t), q columns in (c,r)-permuted order
    maskf_d = nc.dram_tensor("maskf", [16, 128, S], F16, kind="ExternalInput")
    out_d = nc.dram_tensor("out", [CORE_ROWS, D], F32, kind="ExternalOutput")

    with tile.TileContext(nc) as tc:
        with tc.tile_pool(name="persist", bufs=1) as persist:
            qt_all = persist.tile([128, 2 * S], F16, tag="qt", name="qt")
            kt_all = persist.tile([128, 2 * S], F16, tag="kt", name="kt")
            vaug = [persist.tile([128, 2048], F16, tag=f"vaug{p}", name=f"vaug{p}")
                    for p in range(N_PAIRS)]
            # stack2[p]: [64tp+d, 512qh+128tt+r] = 16*O^T[d, q''] with
            # q'' = 1024qh + 128(2tt+tp) + r, tt in [0,4)
            stack2 = [persist.tile([128, 1024], F16, tag=f"stk{p}", name=f"stk{p}")
                      for p in range(N_PAIRS)]
            wo_sb = persist.tile([128, 8192], F16, tag="wo", name="wo")
            maskf_sb = [persist.tile([128, S], F16, tag=f"mask{t}", name=f"mask{t}")
                        for t in range(16)]

            # ---------------- Phase 1: projections ----------------
            with tc.tile_pool(name="asm_sb", bufs=1) as asmp:
                qsb = [asmp.tile([128, 1536], F16, tag=f"qsb{j}", name=f"qsb{j}")
                       for j in range(8)]
                ksb = [asmp.tile([128, 1536], F16, tag=f"ksb{j}", name=f"ksb{j}")
                       for j in range(8)]
                vsb = [asmp.tile([128, 1536], F16, tag=f"vsb{j}", name=f"vsb{j}")
                       for j in range(8)]
                # DMA priority: q (PE starts on chunk 0), k, early masks,
                # v, wo, remaining masks.
                qeng = (nc.sync, nc.scalar, nc.gpsimd)
                for j in range(8):
                    qeng[j % 3].dma_start(out=qsb[j][:, :], in_=qasm[j])
                for j in range(8):
                    qeng[(j + 1) % 3].dma_start(out=ksb[j][:, :], in_=kasm[j])
                # masks go on the gpsimd queue only: HWDGE dma_starts on
                # sync/scalar block those queues until the transfer lands,
                # which would stall the scatter casts (and with them the
                # psum recycling that K proj needs)
                # gate the gpsimd DMA batch on the last k chunk so q/k get
                # the full DMA bus while the PE is consuming them; v isn't
                # needed until ~50us, masks until the attention stream
                scr = asmp.tile([128, 8], F16, tag="scr", name="scr")
                nc.gpsimd.tensor_copy(scr[:, :], ksb[7][:, 1528:1536])
                for j in range(8):
                    nc.gpsimd.dma_start(out=vsb[j][:, :], in_=vasm[j])
                nc.sync.dma_start(out=wo_sb[:, :], in_=wodr[:, :])
                for t in range(16):
                    nc.gpsimd.dma_start(out=maskf_sb[t][:, :], in_=maskf_d[t])
                for p in range(N_PAIRS):
                    va3 = vaug[p][:, :].rearrange("p (t c) -> p t c", c=128)
                    nc.gpsimd.memset(va3[:, :, 0:64], 1.0)

                def scatter_qk(ps, dst_all, tt, engines):
                    # scatter-cast: psum[64sub+d, 256g+128hp+r]
                    #   -> dst[64hp+d, 2048g+128(2tt+sub)+r]
                    dst4 = dst_all[:, :].rearrange(
                        "p (g t r) -> p g t r", g=2, t=16)
                    i = 0
                    for sub in range(2):
                        s4 = ps[64 * sub:64 * (sub + 1), :].rearrange(
                            "p (g h r) -> p g h r", g=2, h=2)
                        for hp in range(2):
                            d_ap = dst4[64 * hp:64 * (hp + 1), :, 2 * tt + sub, :]
                            s_ap = s4[:, :, hp, :]
                            eng = engines[i % len(engines)]
                            i += 1
                            if eng is nc.scalar:
                                nc.scalar.activation(
                                    d_ap, s_ap,
                                    mybir.ActivationFunctionType.Copy)
                            else:
                                eng.tensor_copy(d_ap, s_ap)

                with tc.tile_pool(name="qk_ps", bufs=8, space="PSUM") as qkps:
                    # Q: j-outer over 8 concurrent psum banks -- the first
                    # matmul depends only on the first DMA chunk
                    # Q/K: j-outer (tolerates per-chunk DMA arrival) in two
                    # 4-tile psum waves each, so wave w's scatter overlaps
                    # wave w+1's matmuls and K's psum reuse never waits on
                    # the whole Q scatter
                    psq = [qkps.tile([128, 512], F32, tag="qk", name=f"psq{tt}")
                           for tt in range(8)]
                    for w in range(2):
                        for j in range(8):
                            for tt in range(4 * w, 4 * w + 4):
                                nc.tensor.matmul(
                                    psq[tt][:, :],
                                    lhsT=qsb[j][:, 128 * tt:128 * (tt + 1)],
                                    rhs=qsb[j][:, 1024:1536],
                                    start=(j == 0), stop=(j == 7))
                                if j == 7:
                                    scatter_qk(psq[tt], qt_all, tt,
                                               (nc.vector, nc.scalar))
                    psk = [qkps.tile([128, 512], F32, tag="qk", name=f"psk{tt}")
                           for tt in range(8)]
                    for w in range(2):
                        for j in range(8):
                            for tt in range(4 * w, 4 * w + 4):
                                nc.tensor.matmul(
                                    psk[tt][:, :],
                                    lhsT=ksb[j][:, 128 * tt:128 * (tt + 1)],
                                    rhs=ksb[j][:, 1024:1536],
                                    start=(j == 0), stop=(j == 7))
                                if j == 7:
                                    scatter_qk(psk[tt], kt_all, tt,
                                               (nc.scalar, nc.vector))

                    # V: (p,oc)-outer so vaug[0] is ready first
                    psv = [qkps.tile([128, 512], F32, tag="qk",
                                     name=f"psv{i}") for i in range(8)]
                    for p in range(N_PAIRS):
                        d3 = vaug[p][:, :].rearrange("p (t c) -> p t c", c=128)
                        for oc in range(2):
                            for j in range(8):
                                nc.tensor.matmul(
                                    psv[2 * p + oc][:, :],
                                    lhsT=vsb[j][:, 1024 + 128 * p:1024 + 128 * (p + 1)],
                                    rhs=vsb[j][:, 512 * oc:512 * (oc + 1)],
                                    start=(j == 0), stop=(j == 7))
                            s3 = psv[2 * p + oc][:, :].rearrange(
                                "p (t c) -> p t c", c=64)
                            nc.vector.tensor_copy(
                                d3[:, 8 * oc:8 * (oc + 1), 64:128], s3)

            # ---------------- Phase 2: attention + output ----------------
            with tc.tile_pool(name="praw_p", bufs=3) as ppool, \
                 tc.tile_pool(name="pm_p", bufs=6) as pmpool, \
                 tc.tile_pool(name="norm", bufs=2) as npool, \
                 tc.tile_pool(name="outc", bufs=2) as opool, \
                 tc.tile_pool(name="st_ps", bufs=3, space="PSUM") as stps, \
                 tc.tile_pool(name="o_ps", bufs=1, space="PSUM") as ops:

                wo3 = wo_sb[:, :].rearrange("p (tt x) -> p tt x", tt=8)

                def emit_part(p, psF, part):
                    # 4 of the 16 wo matmuls (part in 0..3 = (qh, tt-pair))
                    qh, th = part // 2, part % 2
                    for tt in (2 * th, 2 * th + 1):
                        TT = 4 * qh + tt   # global t-pair = t//2
                        for oc in range(2):
                            nc.tensor.matmul(
                                psF[:, 512 * oc:512 * (oc + 1)],
                                lhsT=stack2[p][:, 512 * qh + 128 * tt:
                                               512 * qh + 128 * (tt + 1)],
                                rhs=wo3[:, TT, 512 * oc:512 * (oc + 1)],
                                start=(part == 0 and tt == 0),
                                stop=(part == 3 and tt == 3))
                    if part == 3:
                        # wodr carries 1/WSCALE so psF is final; copy+DMA
                        # in halves so the first DMA overlaps the second copy
                        osb = opool.tile([128, 1024], F32, tag="osb", name="osb")
                        nc.vector.tensor_copy(osb[:, 0:512], psF[:, 0:512])
                        nc.sync.dma_start(out=out_d[128 * p:128 * (p + 1), 0:512],
                                          in_=osb[:, 0:512])
                        nc.vector.tensor_copy(osb[:, 512:1024], psF[:, 512:1024])
                        nc.scalar.dma_start(
                            out=out_d[128 * p:128 * (p + 1), 512:1024],
                            in_=osb[:, 512:1024])

                pending_emit = [None]

                for p in range(N_PAIRS):
                    g, hp = p // 2, p % 2
                    lo, hi = 64 * hp, 64 * (hp + 1)
                    for qh in range(2):
                        psO = ops.tile([128, 1024], F32, tag="o", name="psO")
                        queue = []

                        def drain_one():
                            t, pm = queue.pop(0)
                            for sc in range(2):
                                nc.tensor.matmul(
                                    psO[:, 512 * sc:512 * (sc + 1)],
                                    lhsT=vaug[p][:, 128 * t:128 * (t + 1)],
                                    rhs=pm[:, 512 * sc:512 * (sc + 1)],
                                    start=(t == 0), stop=(t == 15))

                        for t in range(16):
                            stt = stps.tile([128, 1024], F32, tag="st", name="stt")
                            for sc in range(2):
                                nc.tensor.matmul(
                                    stt[:, 512 * sc:512 * (sc + 1)],
                                    lhsT=kt_all[lo:hi,
                                                2048 * g + 128 * t:2048 * g + 128 * (t + 1)],
                                    rhs=qt_all[lo:hi,
                                               2048 * g + 1024 * qh + 512 * sc:
                                               2048 * g + 1024 * qh + 512 * (sc + 1)],
                                    start=True, stop=True)
                            praw = ppool.tile([128, 1024], F16, tag="praw", name="praw")
                            nc.scalar.activation(praw[:, :], stt[:, :],
                                                 mybir.ActivationFunctionType.Exp,
                                                 scale=EXP_SCALE)
                            pm = pmpool.tile([128, 1024], F16, tag="pm", name="pm")
                            nc.vector.tensor_mul(pm[:, :], praw[:, :],
                                                 maskf_sb[t][:, 1024 * qh:1024 * (qh + 1)])
                            queue.append((t, pm))
                            if p == 3 and qh == 1 and t == 10:
                                # last pair: its qh0 stack2 half is already
                                # written, so emit those 8 wo matmuls inside
                                # this block; only qh1's 8 remain for the
                                # tail.  psF allocated late to limit stt
                                # pool starvation.
                                pending_emit[0] = (3, stps.tile(
                                    [128, 1024], F32, tag="st", name="psF"),
                                    0, {11: 0, 13: 1})
                            if pending_emit[0] is not None:
                                ep, epsF, done, sched = pending_emit[0]
                                # spread the 16 wo matmuls as bursts of 4
                                # so no single burst stalls the exp cadence
                                if sched.get(t) == done:
                                    emit_part(ep, epsF, done)
                                    pending_emit[0] = (ep, epsF, done + 1,
                                                       sched)
                                    if done + 1 == 4:
                                        pending_emit[0] = None
                            if len(queue) > 2:
                                drain_one()
                        while queue:
                            drain_one()

                        # psO[0:64] = den copies, psO[64:128] = 16*O^T
                        recip = npool.tile([64, 1024], F32, tag="rc", name="recip")
                        nc.vector.reciprocal_approx_fast(recip[:, :], psO[0:64, :])
                        tmpn = npool.tile([128, 1024], F16, tag="tn", name="tmpn")
                        nc.vector.tensor_mul(tmpn[64:128, :], psO[64:128, :],
                                             recip[:, :])
                        # repack to stack2: even t -> partitions 0:64,
                        # odd t -> 64:128; cols compress 128tq'+r -> 128tt+r
                        src3 = tmpn[64:128, :].rearrange(
                            "p (tt tp r) -> p tt tp r", tt=4, tp=2)
                        for tp in range(2):
                            nc.vector.tensor_copy(
                                stack2[p][64 * tp:64 * (tp + 1),
                                          512 * qh:512 * (qh + 1)],
                                src3[:, :, tp, :])
                        if qh == 1 and p < 3:
                            pending_emit[0] = (p, stps.tile([128, 1024], F32,
                                                            tag="st", name="psF"),
                                               0, {2: 0, 4: 1, 6: 2, 8: 3})
                if pending_emit[0] is not None:
                    ep, epsF, done, _ = pending_emit[0]
                    for part in range(done, 4):
                        emit_part(ep, epsF, part)

    nc.finalize()
    return nc


def build_in_maps(inputs):
    q = np.asarray(inputs["q"], dtype=np.float32)
    k = np.asarray(inputs["k"], dtype=np.float32)
    v = np.asarray(inputs["v"], dtype=np.float32)
    mask = np.asarray(inputs["mask"])
    w_q = np.asarray(inputs["w_q"], dtype=np.float32)
    w_k = np.asarray(inputs["w_k"], dtype=np.float32)
    w_v = np.asarray(inputs["w_v"], dtype=np.float32)
    w_o = np.asarray(inputs["w_o"], dtype=np.float32)

    wqT = np.ascontiguousarray(w_q.T) * WSCALE
    wkT = np.ascontiguousarray(w_k.T) * WSCALE
    wvT = np.ascontiguousarray(w_v.T) * WSCALE
    # 1/WSCALE (not WSCALE): folds the 1/WSCALE^2 rescale of the
    # wv*wo WSCALE factors into the weights, so psF needs no rescale
    wo16 = np.ascontiguousarray(w_o.T) * (1.0 / WSCALE)  # [dm, c']
    # wodr[64tp+d, 1024tt + c'] = wo16[64(2tt+tp)+d, c']
    wodr = np.ascontiguousarray(
        wo16.reshape(8, 2, 64, D).transpose(1, 2, 0, 3).reshape(128, 8 * D)
    ).astype(np.float16)

    maskf = []
    for b in range(B):
        mt = (~mask[b]).T.astype(np.float16) * np.float16(MASK_SHIFT)
        mp = mt.reshape(S, 128, 16).transpose(0, 2, 1).reshape(S, S)
        maskf.append(np.ascontiguousarray(
            mp.reshape(128, 16, S).transpose(1, 0, 2)))

    in_maps = []
    for c in range(N_CORES):
        b, sb = c // 4, c % 4
        rows = slice(CORE_ROWS * sb, CORE_ROWS * (sb + 1))
        xqT = np.ascontiguousarray(q[b, rows].T)
        xkT = np.ascontiguousarray(k[b, rows].T)
        xvT = np.ascontiguousarray(v[b, rows].T)

        def pack(wT, xT):
            wc = wT.reshape(8, 128, D)
            xc = xT.reshape(8, 128, CORE_ROWS)
            return np.ascontiguousarray(
                np.concatenate([wc, xc], axis=2)).astype(np.float16)

        in_maps.append({
            "qasm": pack(wqT, xqT),
            "kasm": pack(wkT, xkT),
            "vasm": pack(wvT, xvT),
            "wodr": wodr,
            "maskf": maskf[b],
        })
    return in_maps


def kernel(q, k, v, mask, w_q, w_k, w_v, w_o):
    global _NC
    if _NC is None:
        _NC = _build_program()

    in_maps = build_in_maps(dict(q=q, k=k, v=v, mask=mask,
                                 w_q=w_q, w_k=w_k, w_v=w_v, w_o=w_o))
    res = run_bass_kernel_spmd(_NC, in_maps, list(range(N_CORES))).results

    out = np.empty((B, S, D), dtype=np.float32)
    for c in range(N_CORES):
        b, sb = c // 4, c % 4
        out[b, CORE_ROWS * sb:CORE_ROWS * (sb + 1)] = res[c]["out"]
    return out


# revision 28
# speedup vs baseline: 1.0564x; 1.0562x over previous
"""Multi-head attention (raw-reshape variant) on 8 trn2 NeuronCores.

Shapes: B=2, S=2048, D=1024, H=16, dh=64.  The reference uses a raw
reshape (B,S,D)->(B,H,S,dh), so head h only sees projected rows
[128h, 128h+128).  Core c handles b=c//4 and the 4 heads of seq-block
c%4.  No collectives.

All data tensors are fp16: softmax-weight quantization noise passes
through to the output at full strength, so fp8 anywhere costs ~3.6%+
output error -- over the 2e-2 gate.

v4 (from the v3 baseline, ~259-265us; this version measures
~235-260us, run-to-run clock-governor variance is +-20us):
  - mask arrives as fp16 via DMA (pre-scaled by e^-4 on the host);
    kills the 16 DVE u8->fp16 casts that inflated the attention-step
    cadence from ~1.07us to ~1.9us, plus the tile_wait_until hack.
    Mask DMAs MUST stay on the gpsimd queue: HWDGE dma_starts block
    the sync/scalar queues until the transfer lands, which starves
    the scatter casts and with them the psum recycling K proj needs.
  - Q/K projections j-outer (tolerate per-chunk DMA arrival; tt-outer
    stalls waiting for all 8 chunks), V (p,oc)-outer so vaug[0] is
    ready first; scatter casts split across DVE+ACT.
  - wo emit spread as 4 bursts of 4 matmuls at t=2,4,6,8 of the next
    block instead of one 16-matmul burst; the last pair emits its qh0
    half inside its own final block (psF allocated at t=10 to limit
    stt-pool starvation), so only 8 wo matmuls remain in the tail.
  - PV drains at lag 3 (not 5): shorter end-of-block drain, ~1.2us
    slack over the exp->mul chain.
  - wodr carries 1/WSCALE (not WSCALE), folding the 1/256 rescale into
    the weights: the final psum->sbuf move is a plain copy, split per
    512-col half so each output DMA overlaps the next copy.
  - gpsimd DMA batch (v + masks) gated on the last k chunk via a tiny
    dependency copy: q/k get the full DMA bus while the PE consumes
    them (v needed ~50us, masks ~66us).
Per-step pipeline unchanged: St = Kt.T @ Qt (K=64) -> exp on ACT ->
mask-mul on DVE -> PV accumulate (K=128, denominator rides along in
the vaug ones-columns); reciprocal+repack on DVE; one wo load.
Known floors per core: PE ~164us at 2.4GHz for the 768 matmuls (the
governor rarely sustains 2.4; St runs K=64 = half-PE inherently),
ACT exp 128 x ~1.11us = 143us.  fp8 anywhere fails the 2e-2 gate
(softmax-weight noise passes through at full strength).
"""

import numpy as np

import concourse.bass as bass
import concourse.mybir as mybir
import concourse.tile as tile
from concourse import bacc
from concourse.bass_utils import run_bass_kernel_spmd

F32 = mybir.dt.float32
F16 = mybir.dt.float16

B, S, D, H, DH = 2, 2048, 1024, 16, 64
N_CORES = 8
CORE_ROWS = 512
N_PAIRS = 4
WSCALE = 16.0
EXP_SCALE = 0.125 / (WSCALE * WSCALE)
MASK_SHIFT = float(np.exp(-4.0))

_NC = None


def _build_program():
    nc = bacc.Bacc()

    # per contraction chunk j: [w chunk (1024 dm) | x chunk (512 rows)]
    qasm = nc.dram_tensor("qasm", [8, 128, 1536], F16, kind="ExternalInput")
    kasm = nc.dram_tensor("kasm", [8, 128, 1536], F16, kind="ExternalInput")
    vasm = nc.dram_tensor("vasm", [8, 128, 1536], F16, kind="ExternalInput")
    wodr = nc.dram_tensor("wodr", [128, 8192], F16, kind="ExternalInput")
    # maskf[t, j, qcol]: fp16 keep-mask * e^-4 for key chunk t (rows
    # k'=16j+t), q columns in (c,r)-permuted order
    maskf_d = nc.dram_tensor("maskf", [16, 128, S], F16, kind="ExternalInput")
    out_d = nc.dram_tensor("out", [CORE_ROWS, D], F32, kind="ExternalOutput")

    with tile.TileContext(nc) as tc:
        with tc.tile_pool(name="persist", bufs=1) as persist:
            qt_all = persist.tile([128, 2 * S], F16, tag="qt", name="qt")
            kt_all = persist.tile([128, 2 * S], F16, tag="kt", name="kt")
            vaug = [persist.tile([128, 2048], F16, tag=f"vaug{p}", name=f"vaug{p}")
                    for p in range(N_PAIRS)]
            # stack2[p]: [64tp+d, 512qh+128tt+r] = 16*O^T[d, q''] with
            # q'' = 1024qh + 128(2tt+tp) + r, tt in [0,4)
            stack2 = [persist.tile([128, 1024], F16, tag=f"stk{p}", name=f"stk{p}")
                      for p in range(N_PAIRS)]
            wo_sb = persist.tile([128, 8192], F16, tag="wo", name="wo")
            maskf_sb = [persist.tile([128, S], F16, tag=f"mask{t}", name=f"mask{t}")
                        for t in range(16)]

            # ---------------- Phase 1: projections ----------------
            with tc.tile_pool(name="asm_sb", bufs=1) as asmp:
                qsb = [asmp.tile([128, 1536], F16, tag=f"qsb{j}", name=f"qsb{j}")
                       for j in range(8)]
                ksb = [asmp.tile([128, 1536], F16, tag=f"ksb{j}", name=f"ksb{j}")
                       for j in range(8)]
                vsb = [asmp.tile([128, 1536], F16, tag=f"vsb{j}", name=f"vsb{j}")
                       for j in range(8)]
                # DMA priority: q (PE starts on chunk 0), k, early masks,
                # v, wo, remaining masks.
                for j in range(8):
                    (nc.sync if j % 2 == 0 else nc.scalar).dma_start(
                        out=qsb[j][:, :], in_=qasm[j])
                for j in range(8):
                    (nc.sync if j % 2 == 1 else nc.scalar).dma_start(
                        out=ksb[j][:, :], in_=kasm[j])
                # masks go on the gpsimd queue only: HWDGE dma_starts on
                # sync/scalar block those queues until the transfer lands,
                # which would stall the scatter casts (and with them the
                # psum recycling that K proj needs)
                # gate the gpsimd DMA batch on the last k chunk so q/k get
                # the full DMA bus while the PE is consuming them; v isn't
                # needed until ~50us, masks until the attention stream
                scr = asmp.tile([128, 8], F16, tag="scr", name="scr")
                nc.gpsimd.tensor_copy(scr[:, :], ksb[7][:, 1528:1536])
                for j in range(8):
                    nc.gpsimd.dma_start(out=vsb[j][:, :], in_=vasm[j])
                nc.sync.dma_start(out=wo_sb[:, :], in_=wodr[:, :])
                for t in range(16):
                    nc.gpsimd.dma_start(out=maskf_sb[t][:, :], in_=maskf_d[t])
                for p in range(N_PAIRS):
                    va3 = vaug[p][:, :].rearrange("p (t c) -> p t c", c=128)
                    nc.gpsimd.memset(va3[:, :, 0:64], 1.0)

                def scatter_qk(ps, dst_all, tt, engines):
                    # scatter-cast: psum[64sub+d, 256g+128hp+r]
                    #   -> dst[64hp+d, 2048g+128(2tt+sub)+r]
                    dst4 = dst_all[:, :].rearrange(
                        "p (g t r) -> p g t r", g=2, t=16)
                    i = 0
                    for sub in range(2):
                        s4 = ps[64 * sub:64 * (sub + 1), :].rearrange(
                            "p (g h r) -> p g h r", g=2, h=2)
                        for hp in range(2):
                            d_ap = dst4[64 * hp:64 * (hp + 1), :, 2 * tt + sub, :]
                            s_ap = s4[:, :, hp, :]
                            eng = engines[i % len(engines)]
                            i += 1
                            if eng is nc.scalar:
                                nc.scalar.activation(
                                    d_ap, s_ap,
                                    mybir.ActivationFunctionType.Copy)
                            else:
                                eng.tensor_copy(d_ap, s_ap)

                with tc.tile_pool(name="qk_ps", bufs=8, space="PSUM") as qkps:
                    # Q: j-outer over 8 concurrent psum banks -- the first
                    # matmul depends only on the first DMA chunk
                    psq = [qkps.tile([128, 512], F32, tag="qk", name=f"psq{tt}")
                           for tt in range(8)]
                    for j in range(8):
                        for tt in range(8):
                            nc.tensor.matmul(
                                psq[tt][:, :],
                                lhsT=qsb[j][:, 128 * tt:128 * (tt + 1)],
                                rhs=qsb[j][:, 1024:1536],
                                start=(j == 0), stop=(j == 7))
                            if j == 7:
                                scatter_qk(psq[tt], qt_all, tt,
                                           (nc.vector, nc.scalar))
                    # K: j-outer like Q (tolerates per-chunk DMA arrival;
                    # tt-outer needs all 8 chunks up front and stalls)
                    psk = [qkps.tile([128, 512], F32, tag="qk", name=f"psk{tt}")
                           for tt in range(8)]
                    for j in range(8):
                        for tt in range(8):
                            nc.tensor.matmul(
                                psk[tt][:, :],
                                lhsT=ksb[j][:, 128 * tt:128 * (tt + 1)],
                                rhs=ksb[j][:, 1024:1536],
                                start=(j == 0), stop=(j == 7))
                            if j == 7:
                                scatter_qk(psk[tt], kt_all, tt,
                                           (nc.scalar, nc.vector))

                    # V: (p,oc)-outer so vaug[0] is ready first
                    psv = [qkps.tile([128, 512], F32, tag="qk",
                                     name=f"psv{i}") for i in range(8)]
                    for p in range(N_PAIRS):
                        d3 = vaug[p][:, :].rearrange("p (t c) -> p t c", c=128)
                        for oc in range(2):
                            for j in range(8):
                                nc.tensor.matmul(
                                    psv[2 * p + oc][:, :],
                                    lhsT=vsb[j][:, 1024 + 128 * p:1024 + 128 * (p + 1)],
                                    rhs=vsb[j][:, 512 * oc:512 * (oc + 1)],
                                    start=(j == 0), stop=(j == 7))
                            s3 = psv[2 * p + oc][:, :].rearrange(
                                "p (t c) -> p t c", c=64)
                            nc.vector.tensor_copy(
                                d3[:, 8 * oc:8 * (oc + 1), 64:128], s3)

            # ---------------- Phase 2: attention + output ----------------
            with tc.tile_pool(name="praw_p", bufs=3) as ppool, \
                 tc.tile_pool(name="pm_p", bufs=6) as pmpool, \
                 tc.tile_pool(name="norm", bufs=2) as npool, \
                 tc.tile_pool(name="outc", bufs=2) as opool, \
                 tc.tile_pool(name="st_ps", bufs=3, space="PSUM") as stps, \
                 tc.tile_pool(name="o_ps", bufs=1, space="PSUM") as ops:

                wo3 = wo_sb[:, :].rearrange("p (tt x) -> p tt x", tt=8)

                def emit_part(p, psF, part):
                    # 4 of the 16 wo matmuls (part in 0..3 = (qh, tt-pair))
                    qh, th = part // 2, part % 2
                    for tt in (2 * th, 2 * th + 1):
                        TT = 4 * qh + tt   # global t-pair = t//2
                        for oc in range(2):
                            nc.tensor.matmul(
                                psF[:, 512 * oc:512 * (oc + 1)],
                                lhsT=stack2[p][:, 512 * qh + 128 * tt:
                                               512 * qh + 128 * (tt + 1)],
                                rhs=wo3[:, TT, 512 * oc:512 * (oc + 1)],
                                start=(part == 0 and tt == 0),
                                stop=(part == 3 and tt == 3))
                    if part == 3:
                        # wodr carries 1/WSCALE so psF is final; copy+DMA
                        # in halves so the first DMA overlaps the second copy
                        osb = opool.tile([128, 1024], F32, tag="osb", name="osb")
                        nc.vector.tensor_copy(osb[:, 0:512], psF[:, 0:512])
                        nc.sync.dma_start(out=out_d[128 * p:128 * (p + 1), 0:512],
                                          in_=osb[:, 0:512])
                        nc.vector.tensor_copy(osb[:, 512:1024], psF[:, 512:1024])
                        nc.scalar.dma_start(
                            out=out_d[128 * p:128 * (p + 1), 512:1024],
                            in_=osb[:, 512:1024])

                pending_emit = [None]

                for p in range(N_PAIRS):
                    g, hp = p // 2, p % 2
                    lo, hi = 64 * hp, 64 * (hp + 1)
                    for qh in range(2):
                        psO = ops.tile([128, 1024], F32, tag="o", name="psO")
                        queue = []

                        def drain_one():
                            t, pm = queue.pop(0)
                            for sc in range(2):
                                nc.tensor.matmul(
                                    psO[:, 512 * sc:512 * (sc + 1)],
                                    lhsT=vaug[p][:, 128 * t:128 * (t + 1)],
                                    rhs=pm[:, 512 * sc:512 * (sc + 1)],
                                    start=(t == 0), stop=(t == 15))

                        for t in range(16):
                            stt = stps.tile([128, 1024], F32, tag="st", name="stt")
                            for sc in range(2):
                                nc.tensor.matmul(
                                    stt[:, 512 * sc:512 * (sc + 1)],
                                    lhsT=kt_all[lo:hi,
                                                2048 * g + 128 * t:2048 * g + 128 * (t + 1)],
                                    rhs=qt_all[lo:hi,
                                               2048 * g + 1024 * qh + 512 * sc:
                                               2048 * g + 1024 * qh + 512 * (sc + 1)],
                                    start=True, stop=True)
                            praw = ppool.tile([128, 1024], F16, tag="praw", name="praw")
                            nc.scalar.activation(praw[:, :], stt[:, :],
                                                 mybir.ActivationFunctionType.Exp,
                                                 scale=EXP_SCALE)
                            pm = pmpool.tile([128, 1024], F16, tag="pm", name="pm")
                            nc.vector.tensor_mul(pm[:, :], praw[:, :],
                                                 maskf_sb[t][:, 1024 * qh:1024 * (qh + 1)])
                            queue.append((t, pm))
                            if p == 3 and qh == 1 and t == 10:
                                # last pair: its qh0 stack2 half is already
                                # written, so emit those 8 wo matmuls inside
                                # this block; only qh1's 8 remain for the
                                # tail.  psF allocated late to limit stt
                                # pool starvation.
                                pending_emit[0] = (3, stps.tile(
                                    [128, 1024], F32, tag="st", name="psF"),
                                    0, {11: 0, 13: 1})
                            if pending_emit[0] is not None:
                                ep, epsF, done, sched = pending_emit[0]
                                # spread the 16 wo matmuls as bursts of 4
                                # so no single burst stalls the exp cadence
                                if sched.get(t) == done:
                                    emit_part(ep, epsF, done)
                                    pending_emit[0] = (ep, epsF, done + 1,
                                                       sched)
                                    if done + 1 == 4:
                                        pending_emit[0] = None
                            if len(queue) > 2:
                                drain_one()
                        while queue:
                            drain_one()

                        # psO[0:64] = den copies, psO[64:128] = 16*O^T
                        recip = npool.tile([64, 1024], F32, tag="rc", name="recip")
                        nc.vector.reciprocal_approx_fast(recip[:, :], psO[0:64, :])
                        tmpn = npool.tile([128, 1024], F16, tag="tn", name="tmpn")
                        nc.vector.tensor_mul(tmpn[64:128, :], psO[64:128, :],
                                             recip[:, :])
                        # repack to stack2: even t -> partitions 0:64,
                        # odd t -> 64:128; cols compress 128tq'+r -> 128tt+r
                        src3 = tmpn[64:128, :].rearrange(
                            "p (tt tp r) -> p tt tp r", tt=4, tp=2)
                        for tp in range(2):
                            nc.vector.tensor_copy(
                                stack2[p][64 * tp:64 * (tp + 1),
                                          512 * qh:512 * (qh + 1)],
                                src3[:, :, tp, :])
                        if qh == 1 and p < 3:
                            pending_emit[0] = (p, stps.tile([128, 1024], F32,
                                                            tag="st", name="psF"),
                                               0, {2: 0, 4: 1, 6: 2, 8: 3})
                if pending_emit[0] is not None:
                    ep, epsF, done, _ = pending_emit[0]
                    for part in range(done, 4):
                        emit_part(ep, epsF, part)

    nc.finalize()
    return nc


def build_in_maps(inputs):
    q = np.asarray(inputs["q"], dtype=np.float32)
    k = np.asarray(inputs["k"], dtype=np.float32)
    v = np.asarray(inputs["v"], dtype=np.float32)
    mask = np.asarray(inputs["mask"])
    w_q = np.asarray(inputs["w_q"], dtype=np.float32)
    w_k = np.asarray(inputs["w_k"], dtype=np.float32)
    w_v = np.asarray(inputs["w_v"], dtype=np.float32)
    w_o = np.asarray(inputs["w_o"], dtype=np.float32)

    wqT = np.ascontiguousarray(w_q.T) * WSCALE
    wkT = np.ascontiguousarray(w_k.T) * WSCALE
    wvT = np.ascontiguousarray(w_v.T) * WSCALE
    # 1/WSCALE (not WSCALE): folds the 1/WSCALE^2 rescale of the
    # wv*wo WSCALE factors into the weights, so psF needs no rescale
    wo16 = np.ascontiguousarray(w_o.T) * (1.0 / WSCALE)  # [dm, c']
    # wodr[64tp+d, 1024tt + c'] = wo16[64(2tt+tp)+d, c']
    wodr = np.ascontiguousarray(
        wo16.reshape(8, 2, 64, D).transpose(1, 2, 0, 3).reshape(128, 8 * D)
    ).astype(np.float16)

    maskf = []
    for b in range(B):
        mt = (~mask[b]).T.astype(np.float16) * np.float16(MASK_SHIFT)
        mp = mt.reshape(S, 128, 16).transpose(0, 2, 1).reshape(S, S)
        maskf.append(np.ascontiguousarray(
            mp.reshape(128, 16, S).transpose(1, 0, 2)))

    in_maps = []
    for c in range(N_CORES):
        b, sb = c // 4, c % 4
        rows = slice(CORE_ROWS * sb, CORE_ROWS * (sb + 1))
        xqT = np.ascontiguousarray(q[b, rows].T)
        xkT = np.ascontiguousarray(k[b, rows].T)
        xvT = np.ascontiguousarray(v[b, rows].T)

        def pack(wT, xT):
            wc = wT.reshape(8, 128, D)
            xc = xT.reshape(8, 128, CORE_ROWS)
            return np.ascontiguousarray(
                np.concatenate([wc, xc], axis=2)).astype(np.float16)

        in_maps.append({
            "qasm": pack(wqT, xqT),
            "kasm": pack(wkT, xkT),
            "vasm": pack(wvT, xvT),
            "wodr": wodr,
            "maskf": maskf[b],
        })
    return in_maps


def kernel(q, k, v, mask, w_q, w_k, w_v, w_o):
    global _NC
    if _NC is None:
        _NC = _build_program()

    in_maps = build_in_maps(dict(q=q, k=k, v=v, mask=mask,
                                 w_q=w_q, w_k=w_k, w_v=w_v, w_o=w_o))
    res = run_bass_kernel_spmd(_NC, in_maps, list(range(N_CORES))).results

    out = np.empty((B, S, D), dtype=np.float32)
    for c in range(N_CORES):
        b, sb = c // 4, c % 4
        out[b, CORE_ROWS * sb:CORE_ROWS * (sb + 1)] = res[c]["out"]
    return out
